# revision 28
# baseline (speedup 1.0000x reference)
"""DAGNN-2021 encoder kernel for Trainium2 (8 NeuronCores, Bass/Tile).

Sharding strategy (per the spec hint):
  - Nodes within each topo level (4096) are sharded 8-ways (512/core).
  - Per level step: each core projects its own z-shard to K|V (fp16),
    AllGathers the full-level KV table (4096x512 fp16) into DRAM, then
    dma_gathers its 512x16 predecessor rows into SBUF in 4 chunks.
  - Attention (scores/softmax/agg) runs on DVE in node-major layout;
    projections run on PE with the hid-major activation as the stationary
    (lhsT) operand so outputs come out node-major; biases are preloaded
    into PSUM with a k=1 ones-matmul.
  - Wo/Wc2 are fused host-side: z = q@Wc1.T + agg@(Wc2@Wo).T + (bc+bo@Wc2.T).
  - The 3 layers run software-pipelined one level apart so each layer's
    KVproj->AllGather->gather chain hides under the other layers' DVE work.

kernel(**inputs) takes the full (unsharded) inputs, returns [65536, 1024] f32.
"""

import sys
from contextlib import ExitStack

for _p in ("/opt/trn_rl_repo",):
    if _p not in sys.path:
        sys.path.insert(0, _p)

import numpy as np

HID = 256
IN_CH = 256
HEADS = 4
HD = 64
LN_EPS = 1e-5
NCORES = 8
N_LEV = 16
M_LEV = 4096
P_PRED = 16
LAYERS = 3


# ---------------------------------------------------------------------------
# device program
# ---------------------------------------------------------------------------

def build_nc(n_lev=N_LEV, layers=LAYERS, m_lev=M_LEV, trace_sim=False):
    import concourse.tile as tile
    from concourse import bacc, mybir

    import concourse.bass as bass

    f32 = mybir.dt.float32
    f16 = mybir.dt.float16
    i16 = mybir.dt.int16
    i32 = mybir.dt.int32
    AF = mybir.ActivationFunctionType
    ALU = mybir.AluOpType
    AX = mybir.AxisListType

    mc = m_lev // NCORES           # nodes per core per level (512)
    nt_n = mc // 128               # node tiles per step (4)
    n_steps = n_lev - 1            # level steps per layer (15)
    nn = n_lev * mc                # own nodes per core (8192)
    E_nt = 128 * P_PRED            # gather idxs per node-tile chunk (2048)
    KV = 2 * HID                   # packed K|V row (512)
    qscale = float(1.0 / np.sqrt(HD))

    nc = bacc.Bacc(
        "TRN2", target_bir_lowering=False, debug=False, num_devices=NCORES,
    )

    # ---- I/O ----
    x_t = nc.dram_tensor("x_t", [2, 128, nn], f16, kind="ExternalInput")
    idx_in = nc.dram_tensor("idx", [n_steps, 128, nt_n * P_PRED], i32,
                            kind="ExternalInput")
    win_t = nc.dram_tensor("win_t", [2, 128, HID], f16, kind="ExternalInput")
    bin_r = nc.dram_tensor("bin_r", [1, HID], f16, kind="ExternalInput")
    wq_t = nc.dram_tensor("wq_t", [layers, 2, 128, HID], f16, kind="ExternalInput")
    wkv_t = nc.dram_tensor("wkv_t", [layers, 2, 128, KV], f16, kind="ExternalInput")
    wc1_t = nc.dram_tensor("wc1_t", [layers, 2, 128, HID], f16, kind="ExternalInput")
    w2o_t = nc.dram_tensor("w2o_t", [layers, 2, 128, HID], f16, kind="ExternalInput")
    bq_r = nc.dram_tensor("bq_r", [layers, 1, HID], f16, kind="ExternalInput")
    bkv_r = nc.dram_tensor("bkv_r", [layers, 1, KV], f16, kind="ExternalInput")
    b2c_r = nc.dram_tensor("b2c_r", [layers, 1, HID], f16, kind="ExternalInput")
    g_rep = nc.dram_tensor("g_rep", [layers, 128, HID], f32, kind="ExternalInput")
    b_rep = nc.dram_tensor("b_rep", [layers, 128, HID], f32, kind="ExternalInput")

    out_c = nc.dram_tensor("out_c", [layers + 1, nn, HID], f16, kind="ExternalOutput")

    h0T_d = nc.dram_tensor("h0T_d", [2, 128, nn], f16)  # internal
    # explicit AllGather bounce buffers (offset-0 APs for indirect gather).
    # Layer 0 has its own; layers 1+2 share one merged buffer (2 slabs of
    # mc rows) so their per-step AllGathers fuse into a single collective.
    ag_in_d = [[nc.dram_tensor(f"agi{l}_{b}", [mc, KV], f16) for b in range(2)]
               for l in range(layers)]
    ag_out_d = [[nc.dram_tensor(f"ago{l}_{b}", [m_lev, KV], f16,
                                addr_space="Shared") for b in range(2)]
                for l in range(layers)]

    with tile.TileContext(nc, trace_sim=trace_sim) as tc, ExitStack() as ctx:
        consts = ctx.enter_context(tc.tile_pool(name="consts", bufs=1))
        work = ctx.enter_context(tc.tile_pool(name="work", bufs=2))
        kvp = ctx.enter_context(tc.tile_pool(name="kvp", bufs=2))
        kvo = ctx.enter_context(tc.tile_pool(name="kvo", bufs=2))
        ztp = ctx.enter_context(tc.tile_pool(name="ztp", bufs=4))
        pcp = ctx.enter_context(tc.tile_pool(name="pcp", bufs=2))
        psA = ctx.enter_context(tc.tile_pool(name="psA", bufs=3, space="PSUM"))
        psT = ctx.enter_context(tc.tile_pool(name="psT", bufs=2, space="PSUM"))
        dram = ctx.enter_context(
            tc.tile_pool(name="dram", bufs=2, space="DRAM"))

        # ---- constants ----
        ident = consts.tile([128, 128], f16)
        nc.gpsimd.memset(ident, 0.0)
        nc.gpsimd.affine_select(
            out=ident, in_=ident, compare_op=ALU.not_equal,
            fill=1.0, base=0, pattern=[[-1, 128]], channel_multiplier=1,
        )
        ones1 = consts.tile([1, 128], f16)
        nc.vector.memset(ones1, 1.0)
        eps_sb = consts.tile([128, 1], f32)
        nc.vector.memset(eps_sb, LN_EPS)

        win_sb = consts.tile([128, 2, HID], f16)
        nc.sync.dma_start(out=win_sb, in_=win_t.ap().transpose([1, 0, 2]))
        bin_sb = consts.tile([1, HID], f16)
        nc.sync.dma_start(out=bin_sb, in_=bin_r.ap())

        wq_sb, wkv_sb, wc1_sb, w2o_sb = [], [], [], []
        bq_sb, bkv_sb, b2c_sb, g16_sb, b16_sb = [], [], [], [], []
        for l in range(layers):
            def _ld(name, shape, dt, src):
                t = consts.tile(shape, dt, tag=f"{name}{l}")
                nc.sync.dma_start(out=t, in_=src)
                return t
            wq_sb.append(_ld("wq", [128, 2, HID], f16, wq_t.ap()[l].transpose([1, 0, 2])))
            wkv_sb.append(_ld("wkv", [128, 2, KV], f16, wkv_t.ap()[l].transpose([1, 0, 2])))
            wc1_sb.append(_ld("wc1", [128, 2, HID], f16, wc1_t.ap()[l].transpose([1, 0, 2])))
            w2o_sb.append(_ld("w2o", [128, 2, HID], f16, w2o_t.ap()[l].transpose([1, 0, 2])))
            bq_sb.append(_ld("bq", [1, HID], f16, bq_r.ap()[l]))
            bkv_sb.append(_ld("bkv", [1, KV], f16, bkv_r.ap()[l]))
            b2c_sb.append(_ld("b2c", [1, HID], f16, b2c_r.ap()[l]))
            g = consts.tile([128, HID], f16, tag=f"g16{l}")
            nc.gpsimd.dma_start(out=g, in_=g_rep.ap()[l])  # SWDGE casts
            g16_sb.append(g)
            b = consts.tile([128, HID], f16, tag=f"b16{l}")
            nc.gpsimd.dma_start(out=b, in_=b_rep.ap()[l])
            b16_sb.append(b)

        out_ap = out_c.ap()
        h0T_ap = h0T_d.ap().transpose([1, 0, 2])  # [128, 2, nn]

        def out_slab(l_out, lv):
            return (out_ap[l_out]
                    .rearrange("(a p) d -> p a d", p=128)
                    [:, lv * nt_n:(lv + 1) * nt_n, :])

        def transpose_to(dst, src16):
            """src16 [128, nt_n, HID] f16 node-major -> dst [128, 2, mc] f16."""
            for hh in range(2):
                pst = psT.tile([128, mc], f16, tag="pst")
                for mt in range(nt_n):
                    nc.tensor.transpose(
                        pst[:, mt * 128:(mt + 1) * 128],
                        src16[:, mt, hh * 128:(hh + 1) * 128],
                        ident,
                    )
                nc.scalar.copy(out=dst[:, hh, :], in_=pst)

        def proj_all(bias_row, groups, n_out=HID):
            """psum [128, nt_n, n_out] = bias + sum_g lhs.T @ w."""
            ps = psA.tile([128, nt_n, n_out], f32, tag="psA")
            for mt in range(nt_n):
                nc.tensor.matmul(ps[:, mt, :], ones1, bias_row[:, :n_out],
                                 start=True, stop=False)
                for gi, (lhs, w) in enumerate(groups):
                    for kt in range(2):
                        nc.tensor.matmul(
                            ps[:, mt, :],
                            lhs[:, kt, mt * 128:(mt + 1) * 128],
                            w[:, kt, :n_out],
                            start=False,
                            stop=(gi == len(groups) - 1 and kt == 1),
                        )
            return ps

        def load_h0T(ch, tag="pc0"):
            t = pcp.tile([128, 2, mc], f16, tag=tag)
            nc.sync.dma_start(out=t, in_=h0T_ap[:, :, ch * mc:(ch + 1) * mc])
            return t

        # ---------------- prologue: input projection ----------------
        # Chunks are emitted lazily: chunk 0 up front (feeds the initial
        # kv stages), the rest interleaved into the supersteps so the first
        # attention work starts immediately instead of waiting ~16 serial
        # PE->Act->PE chunk chains.
        def emit_h0_chunk(ch):
            xch = work.tile([128, 2, mc], f16, tag="aggT")
            nc.sync.dma_start(
                out=xch,
                in_=x_t.ap().transpose([1, 0, 2])[:, :, ch * mc:(ch + 1) * mc])
            ps = proj_all(bin_sb, [(xch, win_sb)])
            h0h = work.tile([128, nt_n, HID], f16, tag="qh")
            nc.scalar.copy(out=h0h, in_=ps)
            nc.sync.dma_start(out=out_slab(0, ch), in_=h0h)
            h0Tc = ztp.tile([128, 2, mc], f16, tag="zT")
            transpose_to(h0Tc, h0h)
            nc.sync.dma_start(out=h0T_ap[:, :, ch * mc:(ch + 1) * mc], in_=h0Tc)

        # ---------------- KV stage: project + allgather + gathers ----------
        def emit_kv_stage(l, j, zT):
            kvout = kvo.tile([128, nt_n, KV], f16, tag="kvout", bufs=1)
            for oh in range(2):
                ps = psA.tile([128, nt_n, HID], f32, tag="psA")
                for mt in range(nt_n):
                    nc.tensor.matmul(
                        ps[:, mt, :], ones1,
                        bkv_sb[l][:, oh * HID:(oh + 1) * HID],
                        start=True, stop=False)
                    for kt in range(2):
                        nc.tensor.matmul(
                            ps[:, mt, :],
                            zT[:, kt, mt * 128:(mt + 1) * 128],
                            wkv_sb[l][:, kt, oh * HID:(oh + 1) * HID],
                            start=False, stop=(kt == 1))
                nc.scalar.copy(out=kvout[:, :, oh * HID:(oh + 1) * HID], in_=ps)

            ag_in = ag_in_d[l][j % 2].ap()
            ag_out = ag_out_d[l][j % 2].ap()
            nc.sync.dma_start(
                out=ag_in.rearrange("(a p) e -> p a e", p=128), in_=kvout)
            nc.gpsimd.collective_compute(
                "AllGather", ALU.bypass,
                replica_groups=[list(range(NCORES))],
                ins=[ag_in.opt()], outs=[ag_out.opt()],
            )
            idxt = work.tile([128, nt_n * P_PRED], i32, tag="idxt", bufs=2)
            nc.sync.dma_start(out=idxt, in_=idx_in.ap()[j])
            return (ag_out, 0, idxt)

        def flush_ag():
            pass

        # ---------------- attention step ----------------
        def emit_gather_half(src, h):
            """Gather node tiles {2h, 2h+1}'s preds into one [128,32,KV] tile."""
            ag_out, elem_off, idxt = src
            kvg = kvp.tile([128, 2 * P_PRED, KV], f16, tag=f"kvg{h}",
                           bufs=2 if h == 0 else 1)
            for tt in range(2):
                for pp in range(P_PRED):
                    col = (2 * h + tt) * P_PRED + pp
                    nc.gpsimd.indirect_dma_start(
                        out=kvg[:, tt * P_PRED + pp, :],
                        out_offset=None,
                        in_=ag_out,
                        in_offset=bass.IndirectOffsetOnAxis(
                            ap=idxt[:, col:col + 1], axis=0),
                        element_offset=elem_off,
                    )
            return kvg

        TP = 2 * P_PRED  # (tile, pred) groups per half-step (32)

        def emit_attention(l, s, kv_src, p_cT, mid_cb=None):
            ps = proj_all(bq_sb[l], [(p_cT, wq_sb[l])])
            qh = work.tile([128, nt_n, HID], f16, tag="qh")
            nc.scalar.copy(out=qh, in_=ps)

            kvg0 = emit_gather_half(kv_src, 0)
            kvg1 = emit_gather_half(kv_src, 1)

            agg = work.tile([128, nt_n, HID], f16, tag="agg", bufs=2)

            def k_batch(kvg, sl, qsl, eng, prod):
                """scores f16 [128, n, HEADS] for tp slice sl of kvg on eng.

                prod is the matching [128, n, HEADS, HD] scratch slice.
                qsl = (lo, n_tiles) node-tile range matching sl.
                """
                lo, ntt = qsl
                n = ntt * P_PRED
                eng.tensor_mul(
                    prod.rearrange("p t h d -> p t (h d)")
                    .rearrange("p (t q) e -> p t q e", q=P_PRED),
                    kvg[:, sl, :HID].rearrange("p (t q) e -> p t q e",
                                               q=P_PRED),
                    qh[:, lo:lo + ntt, :].unsqueeze(2)
                    .broadcast_to([128, ntt, P_PRED, HID]),
                )
                for w in (32, 16, 8):
                    eng.tensor_add(
                        prod[:, :, :, 0:w], prod[:, :, :, 0:w],
                        prod[:, :, :, w:2 * w])
                scores = work.tile([128, TP, HEADS], f16,
                                   tag="scores", bufs=2)
                eng.tensor_reduce(
                    out=scores[:, 0:n, :], in_=prod[:, :, :, 0:8],
                    axis=AX.X, op=ALU.add)
                return scores[:, 0:n, :]

            def v_half(h, kvg, escx):
                """agg[:, 2h:2h+2, :] from escx (exp-scores pre-seeded at
                [...,0:1]); escx is consumed in place as the V product."""
                for w in (1, 2, 4, 8, 16, 32):
                    nc.vector.tensor_copy(
                        out=escx[:, :, :, w:2 * w], in_=escx[:, :, :, 0:w])
                den = work.tile([128, 2, HEADS], f32, tag="den", bufs=2)
                nc.vector.tensor_reduce(
                    out=den,
                    in_=escx[:, :, :, 0].rearrange("p (t q) h -> p t h q",
                                                   q=P_PRED),
                    axis=AX.X, op=ALU.add)
                rden = work.tile([128, 2, HEADS], f16, tag="rden", bufs=2)
                nc.vector.reciprocal(out=rden, in_=den)
                # escx <- escx * V, then tree-sum over preds
                nc.vector.tensor_mul(
                    escx.rearrange("p t h d -> p t (h d)"),
                    escx.rearrange("p t h d -> p t (h d)"),
                    kvg[:, :, HID:])
                pv = escx.rearrange("p (t q) h d -> p t q (h d)", q=P_PRED)
                for w in (8, 4, 2, 1):
                    nc.vector.tensor_add(
                        pv[:, :, 0:w, :], pv[:, :, 0:w, :],
                        pv[:, :, w:2 * w, :])
                nc.vector.tensor_mul(
                    agg[:, 2 * h:2 * h + 2, :]
                    .rearrange("p t (h d) -> p t h d", h=HEADS),
                    pv[:, :, 0, :].rearrange("p t (h d) -> p t h d", h=HEADS),
                    rden.unsqueeze(3).broadcast_to([128, 2, HEADS, HD]),
                )

            with nc.allow_low_precision(reason="fp16 attention"):
                # the K-dot scratch doubles as the exp(V-weight) buffer:
                # once scores are extracted, exp() is seeded at [...,0:1] and
                # expanded in place, so no separate escx allocation exists.
                prod0 = work.tile([128, TP, HEADS, HD], f16, tag="prod")
                sc0 = k_batch(kvg0, slice(0, TP), (0, 2), nc.vector, prod0)
                nc.scalar.activation(
                    out=prod0[:, :, :, 0:1], in_=sc0.unsqueeze(3), func=AF.Exp)
                v_half(0, kvg0, prod0)
                if mid_cb is not None:
                    mid_cb()   # finish the previous step mid-stream
                prod1 = work.tile([128, TP, HEADS, HD], f16, tag="prod")
                sc1 = k_batch(kvg1, slice(0, TP), (2, 2), nc.vector, prod1)
                nc.scalar.activation(
                    out=prod1[:, :, :, 0:1], in_=sc1.unsqueeze(3), func=AF.Exp)
                v_half(1, kvg1, prod1)
            return agg

        def emit_finish_a(l, s, agg, p_cT):
            """Phase A: PE/Act-side work (transpose, out-proj, LN stats).

            Returns state for emit_finish_b; split so the DVE-side combines
            can be emitted later, after other ready DVE work, instead of
            stalling the DVE stream on the PE->Act chain."""
            aggT = work.tile([128, 2, mc], f16, tag="aggT")
            transpose_to(aggT, agg)

            ps = proj_all(b2c_sb[l], [(p_cT, wc1_sb[l]), (aggT, w2o_sb[l])])

            # LN statistics + normalize run on the Act engine (Square /
            # Identity live in every act table, so no table reloads), with
            # only the tiny [128, nt_n] combines left on DVE.
            sums = work.tile([128, nt_n, 2], f32, tag="sums")
            junk = work.tile([128, HID], f32, tag="junk", bufs=1)
            for mt in range(nt_n):
                nc.scalar.activation(
                    out=junk, in_=ps[:, mt, :], func=AF.Square,
                    accum_out=sums[:, mt, 1:2])
                nc.scalar.activation(
                    out=junk, in_=ps[:, mt, :], func=AF.Identity,
                    accum_out=sums[:, mt, 0:1])
            return ps, sums

        def emit_finish_b(l, s, ps, sums):
            mv = work.tile([128, nt_n, 2], f32, tag="mv")
            nc.vector.tensor_scalar(
                out=mv, in0=sums, scalar1=1.0 / HID, scalar2=None,
                op0=ALU.mult, op1=ALU.bypass)   # [mean, E[x^2]]
            veps = work.tile([128, nt_n, 1], f32, tag="veps")
            nc.vector.scalar_tensor_tensor(
                out=veps, in0=mv[:, :, 0:1], scalar=-1.0,
                in1=mv[:, :, 0:1], op0=ALU.mult, op1=ALU.mult)  # -mean^2
            nc.vector.tensor_scalar(
                out=veps, in0=veps, scalar1=LN_EPS, scalar2=None,
                op0=ALU.add, op1=ALU.bypass)
            nc.vector.tensor_add(veps, veps, mv[:, :, 1:2])  # var + eps
            nc.vector.reciprocal(out=veps, in_=veps)
            rstd = work.tile([128, nt_n, 1], f32, tag="rstd")
            nc.scalar.activation(out=rstd, in_=veps, func=AF.Sqrt)
            nmr = work.tile([128, nt_n, 1], f32, tag="nmr")
            nc.vector.scalar_tensor_tensor(
                out=nmr, in0=mv[:, :, 0:1], scalar=-1.0,
                in1=rstd, op0=ALU.mult, op1=ALU.mult)   # -mean*rstd
            zn16 = work.tile([128, nt_n, HID], f16, tag="zn16", bufs=1)
            for mt in range(nt_n):
                nc.scalar.activation(
                    out=zn16[:, mt, :], in_=ps[:, mt, :], func=AF.Identity,
                    scale=rstd[:, mt, :], bias=nmr[:, mt, :])
            nc.vector.tensor_mul(
                zn16, zn16,
                g16_sb[l].unsqueeze(1).broadcast_to([128, nt_n, HID]))
            nc.vector.tensor_add(
                zn16, zn16,
                b16_sb[l].unsqueeze(1).broadcast_to([128, nt_n, HID]))
            z16 = work.tile([128, nt_n, HID], f16, tag="z16", bufs=1)
            nc.scalar.activation(out=z16, in_=zn16, func=AF.Gelu)

            nc.sync.dma_start(out=out_slab(l + 1, s + 1), in_=z16)

            zT = ztp.tile([128, 2, mc], f16, tag="zT")
            transpose_to(zT, z16)
            return zT

        # ---------------- pipelined supersteps ----------------
        # LN/out-proj of each step is deferred past the next step's attention
        # so the DVE stream always has ready work while gathers land.
        n_steps_l = n_lev - 1
        kvg_next = {}
        zT_last = {}
        emit_h0_chunk(0)
        for l in range(layers):
            kvg_next[l] = emit_kv_stage(l, 0, load_h0T(0, tag="pckv"))
        emit_h0_chunk(1)

        def flush(pending):
            if pending is None:
                return
            fl, fs, fagg, fpcT = pending
            ps, sums = emit_finish_a(fl, fs, fagg, fpcT)
            zT = emit_finish_b(fl, fs, ps, sums)
            zT_last[fl] = zT
            if fs + 1 < n_steps_l:
                kvg_next[fl] = emit_kv_stage(fl, fs + 1, zT)

        pending = None
        for t in range(n_steps_l + layers - 1):
            for l in range(layers - 1, -1, -1):
                s = t - l
                if not (0 <= s < n_steps_l):
                    continue
                if pending is not None and l > 0 and pending[0] == l - 1:
                    flush(pending)   # ramp-up: attention needs pending's zT
                    pending = None
                if l == 0 and s + 2 < n_lev:
                    emit_h0_chunk(s + 2)   # one superstep ahead of its use
                p_cT = load_h0T(s + 1) if l == 0 else zT_last[l - 1]
                fp = pending
                fired = []

                def mid_cb():
                    flush(fp)
                    fired.append(True)
                agg = emit_attention(
                    l, s, kvg_next[l], p_cT,
                    mid_cb=mid_cb if fp is not None else None)
                if fp is not None and not fired:
                    flush(fp)
                pending = (l, s, agg, p_cT)
        flush(pending)
        flush_ag()

    nc.compile()
    return nc


# ---------------------------------------------------------------------------
# host orchestration
# ---------------------------------------------------------------------------

def prep_inputs(x, preds_lv, W_in, b_in, Wq, bq, Wk, bk, Wv, bv, Wo, bo,
                Wc, bc, ln_g, ln_b, n_lev=N_LEV, layers=LAYERS, m_lev=M_LEV):
    f16 = np.float16
    mc = m_lev // NCORES
    n_steps = n_lev - 1
    nn = n_lev * mc

    Wc1 = Wc[:, :, :HID]
    Wc2 = Wc[:, :, HID:]
    W2o = np.einsum("lij,ljk->lik", Wc2, Wo)
    b2c = bc + np.einsum("lij,lj->li", Wc2, bo)
    qscale = 1.0 / np.sqrt(HID // HEADS)
    Wq = Wq * qscale     # fold attention scale into the Q projection
    bq = bq * qscale

    def tr(w):  # [L, out, in] -> [L, 2, 128, out] (W.T tiled on contraction)
        return np.ascontiguousarray(
            w.transpose(0, 2, 1).reshape(w.shape[0], 2, 128, w.shape[1])
        ).astype(f16)

    wkv = np.concatenate([Wk, Wv], axis=1)
    common = dict(
        win_t=np.ascontiguousarray(W_in.T.reshape(2, 128, HID)).astype(f16),
        bin_r=b_in.reshape(1, HID).astype(f16),
        wq_t=tr(Wq), wkv_t=tr(wkv), wc1_t=tr(Wc1), w2o_t=tr(W2o),
        bq_r=bq.reshape(layers, 1, HID).astype(f16),
        bkv_r=np.concatenate([bk, bv], axis=1).reshape(layers, 1, 2 * HID).astype(f16),
        b2c_r=b2c.reshape(layers, 1, HID).astype(f16),
        g_rep=np.ascontiguousarray(
            np.broadcast_to(ln_g[:, None, :], (layers, 128, HID))).astype(np.float32),
        b_rep=np.ascontiguousarray(
            np.broadcast_to(ln_b[:, None, :], (layers, 128, HID))).astype(np.float32),
    )

    x_r = np.asarray(x, np.float32).reshape(n_lev, NCORES, mc, IN_CH)
    preds = np.asarray(preds_lv).reshape(n_steps, NCORES, mc, P_PRED)

    in_maps = []
    for c in range(NCORES):
        xc = x_r[:, c].reshape(nn, IN_CH)
        x_t = np.ascontiguousarray(xc.T.reshape(2, 128, nn)).astype(f16)
        nt_n = mc // 128
        idx = np.empty((n_steps, 128, nt_n * P_PRED), np.int32)
        for j in range(n_steps):
            pl = preds[j, c].astype(np.int64) - j * m_lev
            idx[j] = (pl.reshape(nt_n, 128, P_PRED).transpose(1, 0, 2)
                      .reshape(128, nt_n * P_PRED).astype(np.int32))
        m = dict(common)
        m["x_t"] = x_t
        m["idx"] = idx
        in_maps.append(m)
    return in_maps


def assemble_out(results, n_lev=N_LEV, layers=LAYERS, m_lev=M_LEV):
    mc = m_lev // NCORES
    N = n_lev * m_lev
    out = np.empty((N, (layers + 1) * HID), np.float32)
    for c in range(NCORES):
        oc = np.asarray(results[c]["out_c"]).reshape(layers + 1, n_lev, mc, HID)
        rows = ((np.arange(n_lev) * m_lev + c * mc)[:, None]
                + np.arange(mc)[None, :]).reshape(-1)
        for l in range(layers + 1):
            blk = oc[l]
            if l > 0:
                blk = blk.copy()
                blk[0] = oc[0][0]  # level-0 rows are never updated
            out[rows, l * HID:(l + 1) * HID] = blk.reshape(-1, HID)
    return out


_NC_CACHE = {}


def run(inputs, trace=False, **run_kwargs):
    """Build+run; returns (full_output, BassKernelResults)."""
    from concourse import bass_utils

    x = np.asarray(inputs["x"], np.float32)
    preds_lv = np.asarray(inputs["preds_lv"])
    args = [np.asarray(inputs[k], np.float32) for k in
            ("W_in", "b_in", "Wq", "bq", "Wk", "bk", "Wv", "bv",
             "Wo", "bo", "Wc", "bc", "ln_g", "ln_b")]

    in_maps = prep_inputs(x, preds_lv, *args)

    key = (N_LEV, LAYERS, M_LEV)
    if key not in _NC_CACHE:
        _NC_CACHE[key] = build_nc()
    nc = _NC_CACHE[key]

    res = bass_utils.run_bass_kernel_spmd(
        nc, in_maps, core_ids=list(range(NCORES)), trace=trace, **run_kwargs)
    return assemble_out(res.results), res


def kernel(x, edge_index, nodes_lv, preds_lv, W_in, b_in, Wq, bq, Wk, bk,
           Wv, bv, Wo, bo, Wc, bc, ln_g, ln_b, **_ignored):
    out, _ = run(dict(x=x, preds_lv=preds_lv, W_in=W_in, b_in=b_in, Wq=Wq,
                      bq=bq, Wk=Wk, bk=bk, Wv=Wv, bv=bv, Wo=Wo, bo=bo,
                      Wc=Wc, bc=bc, ln_g=ln_g, ln_b=ln_b))
    return out


if __name__ == "__main__":
    import reference

    inp = reference.setup_inputs()
    inp = {k: np.asarray(v) for k, v in inp.items()}
    got = kernel(**inp)
    exp = np.asarray(reference.reference(**inp))
    err = np.abs(got - exp) / (np.abs(exp) + 1e-5)
    print("max rel err", err.max(), "mean", err.mean())

